# revision 1
# baseline (speedup 1.0000x reference)
"""DTW teacher-feature expansion kernel for Trainium2 (8 NeuronCores, data parallel).

For each of 16 (teacher[400,1024], student[600,1024]) pairs:
  D = pairwise euclidean distance, R = DTW accumulated-cost DP, exact
  backtrack path (argmin over diag/up/left, diag preferred on ties),
  expanded[j] += teacher[i] over path cells. Returns [16,600,1024] f32.

Per core (2 samples):
  1. D = sqrt(relu(aa + bb - 2 t@s^T)) via PE matmuls.
  2. Forward DP row-by-row; the in-row recurrence
     R[j] = min(D[j]+R[j-1], min(U[j],U[j-1])+D[j]) is one
     tensor_tensor_scan(op0=add, op1=min) per row (both samples batched
     on partitions).
  3. Bulk per-cell argmin masks from R kept in a j-partitioned grid plus a
     j-shifted duplicate, so comparisons are same-partition free shifts.
  4. Path indicator W propagated backward row-by-row; the in-row dependency
     W[i,j] = G[i,j] | (left[i,j+1] & W[i,j+1]) is one reversed
     tensor_tensor_scan(op0=logical_and, op1=logical_or) per row.
  5. expanded = W^T @ teacher via PE matmuls (exact: W is 0/1).

W-propagation reproduces the reference backtrack bit-exactly given R; R is
bit-exact given D (min exact; the one fp32 add per cell rounds monotonically).
D differs from the grader's only in matmul reduction order (~1e-6 abs), far
below the empirically measured path-decision margins (~1e-2).
"""
import os
import sys

for _p in ("/opt/trn_rl_repo", "/root/.axon_site/_ro/trn_rl_repo"):
    if os.path.isdir(_p) and _p not in sys.path:
        sys.path.insert(0, _p)

import numpy as np
from contextlib import ExitStack

import concourse.bass as bass
import concourse.bacc as bacc
import concourse.mybir as mybir
from concourse import tile

F32 = mybir.dt.float32
BF16 = mybir.dt.bfloat16
AOT = mybir.AluOpType
ACTF = mybir.ActivationFunctionType

B, T1, T2, DM = 16, 400, 600, 1024
NCORES, SPC = 8, 2
BIG = 1.0e30
KCH = DM // 128              # 8 K-chunks for the D matmul
ICH = (T1 + 127) // 128      # 4 i-chunks: 128,128,128,16
JCH = (T2 + 127) // 128      # 5 j-chunks: 128x4 + 88
JSTR = 2 * JCH               # free stride per i in the j-partitioned grids
GW = (T1 + 1) * JSTR         # grid width: slot(i,jc,s) = (i+1)*JSTR + jc*2 + s
NH = 2                       # n-halves of 300 for the D matmul
MT = 120                     # M-tile of the output matmul (600 = 5*120)
TAIL = T2 - 128 * (JCH - 1)  # 88


def _ich(c):
    return min(128, T1 - 128 * c)


def build_kernel(nc, tT=None, sT=None, tnat=None, out=None, dbg=None):
    if tT is None:
        tT = nc.dram_tensor("tT", [SPC, DM, T1], F32, kind="ExternalInput")
        sT = nc.dram_tensor("sT", [SPC, DM, T2], F32, kind="ExternalInput")
        tnat = nc.dram_tensor("tnat", [SPC, T1, DM], F32, kind="ExternalInput")
        out = nc.dram_tensor("out", [SPC, T2, DM], F32, kind="ExternalOutput")

    with ExitStack() as ctx, tile.TileContext(nc) as tc:
        esD, esR, esM = ExitStack(), ExitStack(), ExitStack()
        with tc.tile_pool(name="pWgp", bufs=1) as pWgp:
            pMp = esM.enter_context(tc.tile_pool(name="pMp", bufs=1))
            pRp = esR.enter_context(tc.tile_pool(name="pRp", bufs=1))
            gpool = esD.enter_context(tc.tile_pool(name="pDp", bufs=1))
            # D rows, i-partitioned: [i%128, (i//128 *2 + s)*600 + j]
            Dg = gpool.tile([128, ICH * SPC * T2], F32, tag="Dg")
            nc.vector.memset(Dg[:, :], 0.0)

            # ---------------- phase 1: D ----------------
            with tc.tile_pool(name="ph1", bufs=1) as p1, \
                 tc.tile_pool(name="pp1", bufs=1, space="PSUM") as pp1:
                ones = p1.tile([128, 1], F32, tag="ones")
                nc.vector.memset(ones[:, :], 1.0)
                for s in range(SPC):
                    tTr = p1.tile([128, KCH * T1], F32, tag="tTr", name="tTr")
                    sTr = p1.tile([128, KCH * T2], F32, tag="sTr", name="sTr")
                    nc.sync.dma_start(
                        tTr[:, :], tT[s, :, :].rearrange("(k p) i -> p k i", p=128))
                    nc.sync.dma_start(
                        sTr[:, :], sT[s, :, :].rearrange("(k p) j -> p k j", p=128))
                    # PE may carry only one sem wait; launder DMA deps via DVE
                    tTa = p1.tile([128, KCH * T1], F32, tag="tTa", name="tTa")
                    sTa = p1.tile([128, KCH * T2], F32, tag="sTa", name="sTa")
                    nc.vector.tensor_copy(out=tTa[:, :], in_=tTr[:, :])
                    nc.vector.tensor_copy(out=sTa[:, :], in_=sTr[:, :])
                    tTt = [tTa[:, k * T1:(k + 1) * T1] for k in range(KCH)]
                    sTt = [sTa[:, k * T2:(k + 1) * T2] for k in range(KCH)]
                    # bb[j] = sum_k s[j,k]^2 via ones-matmul over squared sT
                    ps_bb = [pp1.tile([1, 300], F32, tag=f"psbb{h}", name=f"psbb{h}")
                             for h in range(NH)]
                    for k in range(KCH):
                        sq = p1.tile([128, T2], F32, tag="sq", name="sq", bufs=2)
                        nc.vector.tensor_tensor(
                            out=sq[:, :], in0=sTt[k], in1=sTt[k],
                            op=AOT.mult)
                        for h in range(NH):
                            nc.tensor.matmul(ps_bb[h][:, :], lhsT=ones[:, :],
                                             rhs=sq[:, 300 * h:300 * (h + 1)],
                                             start=(k == 0), stop=(k == KCH - 1))
                    bb_sb = p1.tile([1, T2], F32, tag="bbsb")
                    for h in range(NH):
                        nc.vector.tensor_copy(out=bb_sb[:, 300 * h:300 * (h + 1)],
                                              in_=ps_bb[h][:, :])
                    bb_bc = p1.tile([128, T2], F32, tag="bbbc")
                    nc.gpsimd.partition_broadcast(bb_bc[:, :], bb_sb[:, :])
                    # aa[i] via ones-matmul over squared tTa
                    ps_aa = pp1.tile([1, T1], F32, tag="psaa")
                    for k in range(KCH):
                        sqt = p1.tile([128, T1], F32, tag="sqt", name="sqt", bufs=2)
                        nc.vector.tensor_tensor(
                            out=sqt[:, :], in0=tTt[k], in1=tTt[k], op=AOT.mult)
                        nc.tensor.matmul(ps_aa[:, :], lhsT=ones[:, :],
                                         rhs=sqt[:, :],
                                         start=(k == 0), stop=(k == KCH - 1))
                    aa_sb = p1.tile([1, T1], F32, tag="aasb")
                    nc.vector.tensor_copy(out=aa_sb[:, :], in_=ps_aa[:, :])
                    aa = [p1.tile([128, 1], F32, tag=f"aa{c}", name=f"aa{c}") for c in range(ICH)]
                    for c in range(ICH):
                        h = _ich(c)
                        nc.sync.dma_start(aa[c][0:h, 0:1],
                                          aa_sb[0:1, 128 * c:128 * c + h])
                    # ab then D = sqrt(relu(-2ab + bb + aa))
                    for c in range(ICH):
                        h = _ich(c)
                        for n2 in range(NH):
                            ps_ab = pp1.tile([128, 300], F32, tag="psab")
                            for k in range(KCH):
                                nc.tensor.matmul(
                                    ps_ab[0:h, :],
                                    lhsT=tTt[k][:, 128 * c:128 * c + h],
                                    rhs=sTt[k][:, 300 * n2:300 * (n2 + 1)],
                                    start=(k == 0), stop=(k == KCH - 1))
                            u = p1.tile([128, 300], F32, tag="u")
                            nc.vector.scalar_tensor_tensor(
                                out=u[0:h, :], in0=ps_ab[0:h, :], scalar=-2.0,
                                in1=bb_bc[0:h, 300 * n2:300 * (n2 + 1)],
                                op0=AOT.mult, op1=AOT.add)
                            nc.vector.tensor_scalar(
                                out=u[0:h, :], in0=u[0:h, :],
                                scalar1=aa[c][0:h, 0:1], scalar2=0.0,
                                op0=AOT.add, op1=AOT.max)
                            db = (c * SPC + s) * T2 + 300 * n2
                            nc.scalar.activation(
                                out=Dg[0:h, db:db + 300], in_=u[0:h, :],
                                func=ACTF.Sqrt)

            # ---------------- phase 2: forward DP (row-staged) ----------------
            # Rr / RrU: i-partitioned row grids, pitch 601 per (c,s) block:
            # slot(i, j, s) = (c*SPC+s)*601 + 1 + j  at partition i%128.
            # RrU holds R[i-1, *] at row i's slot (vertical-shift duplicate),
            # so every mask comparison is a same-partition free-dim shift.
            RS = 4
            NG = T1 // RS
            GW2 = ICH * SPC * 601
            Rr = pRp.tile([128, GW2], F32, tag="Rr")
            RrU = pRp.tile([128, GW2], F32, tag="RrU")
            nc.vector.memset(Rr[:, :], BIG)
            nc.vector.memset(RrU[:, :], BIG)

            def blk(c, s):
                return (c * SPC + s) * 601 + 1

            with tc.tile_pool(name="ph2", bufs=1) as p2, \
                 tc.tile_pool(name="pdr", bufs=2) as pdr, \
                 tc.tile_pool(name="prr", bufs=2) as prr:
                bigrow = p2.tile([SPC, T2], F32, tag="bigrow")
                nc.vector.memset(bigrow[:, :], BIG)
                prev = None
                for g in range(NG):
                    i0 = g * RS
                    c, p0 = i0 // 128, i0 % 128
                    ds_ = pdr.tile([SPC, RS * T2], F32, tag="ds", name="ds")
                    for s in range(SPC):
                        nc.sync.dma_start(
                            ds_[s:s + 1, :],
                            Dg[p0:p0 + RS, (c * SPC + s) * T2:(c * SPC + s + 1) * T2])
                    rs_ = prr.tile([SPC, RS * (T2 + 1)], F32, tag="rs", name="rs")
                    for r in range(RS):
                        i = i0 + r
                        rb = r * (T2 + 1)
                        dr = ds_[:, r * T2:(r + 1) * T2]
                        rr = rs_[:, rb:rb + T2 + 1]
                        nc.vector.memset(rs_[:, rb:rb + 1], BIG)
                        if i == 0:
                            nc.vector.tensor_tensor_scan(
                                out=rr[:, 1:T2 + 1], data0=dr, data1=bigrow[:, :],
                                initial=0.0, op0=AOT.add, op1=AOT.min)
                        else:
                            m = p2.tile([SPC, T2], F32, tag="m")
                            nc.vector.tensor_tensor(
                                out=m[:, :], in0=prev[:, 1:T2 + 1], in1=prev[:, 0:T2],
                                op=AOT.min)
                            nc.vector.tensor_tensor(
                                out=m[:, :], in0=m[:, :], in1=dr, op=AOT.add)
                            nc.vector.tensor_tensor_scan(
                                out=rr[:, 1:T2 + 1], data0=dr, data1=m[:, :],
                                initial=BIG, op0=AOT.add, op1=AOT.min)
                        prev = rr
                    # contiguous row-major stores: rows -> Rr, rows -> RrU
                    # (shifted one grid row down)
                    for s in range(SPC):
                        rv = rs_[s:s + 1, :].rearrange(
                            "a (r c2) -> a r c2", r=RS)[:, :, 1:T2 + 1]
                        nc.sync.dma_start(
                            Rr[p0:p0 + RS, blk(c, s):blk(c, s) + T2].opt(),
                            rv.opt())
                        iu = i0 + 1
                        cu_, pu = iu // 128, iu % 128
                        if pu + RS <= 128:
                            nc.sync.dma_start(
                                RrU[pu:pu + RS, blk(cu_, s):blk(cu_, s) + T2].opt(),
                                rv.opt())
                        else:
                            n1 = 128 - pu
                            nc.sync.dma_start(
                                RrU[pu:128, blk(cu_, s):blk(cu_, s) + T2].opt(),
                                rv[:, 0:n1, :].opt())
                            nc.sync.dma_start(
                                RrU[0:RS - n1,
                                    blk(cu_ + 1, s):blk(cu_ + 1, s) + T2].opt(),
                                rv[:, n1:RS, :].opt())

            if dbg is not None:
                nc.sync.dma_start(dbg["Dg"][:, :], Dg[:, :])
                nc.sync.dma_start(dbg["Rg0"][:, :], Rr[:, :])
            esD.close()   # free Dg + phase-1/2 space

            # ---------------- phase 3: bulk choice masks ----------------
            # cell o (interior): cu = RrU[o], cd = RrU[o-1], cl = Rr[o-1]
            Mdg = pMp.tile([128, GW2], BF16, tag="Mdg")
            Mug = pMp.tile([128, GW2], BF16, tag="Mug")
            Mlg = pMp.tile([128, GW2], BF16, tag="Mlg")
            nc.vector.memset(Mdg[:, :], 0.0)
            nc.vector.memset(Mug[:, :], 0.0)
            nc.vector.memset(Mlg[:, :], 0.0)
            with tc.tile_pool(name="ph3", bufs=1) as p3:
                N = GW2 - 1
                cu = RrU[:, 1:GW2]
                cd = RrU[:, 0:N]
                cl = Rr[:, 0:N]
                w1 = p3.tile([128, N], BF16, tag="w1")
                w2 = p3.tile([128, N], BF16, tag="w2")
                dgm = Mdg[:, 1:GW2]
                ugm = Mug[:, 1:GW2]
                lgm = Mlg[:, 1:GW2]
                nc.vector.tensor_tensor(out=w1[:, :], in0=cd, in1=cu, op=AOT.is_le)
                nc.vector.tensor_tensor(out=w2[:, :], in0=cd, in1=cl, op=AOT.is_le)
                nc.vector.tensor_tensor(out=dgm, in0=w1[:, :], in1=w2[:, :],
                                        op=AOT.logical_and)
                nc.vector.tensor_tensor(out=w1[:, :], in0=cu, in1=cl, op=AOT.is_le)
                nc.vector.tensor_scalar(out=w2[:, :], in0=dgm, scalar1=-1.0,
                                        scalar2=1.0, op0=AOT.mult, op1=AOT.add)
                nc.vector.tensor_tensor(out=ugm, in0=w2[:, :], in1=w1[:, :],
                                        op=AOT.logical_and)
                nc.vector.tensor_scalar(out=w1[:, :], in0=w1[:, :], scalar1=-1.0,
                                        scalar2=1.0, op0=AOT.mult, op1=AOT.add)
                nc.vector.tensor_tensor(out=lgm, in0=w2[:, :], in1=w1[:, :],
                                        op=AOT.logical_and)
            esR.close()   # free Rr/RrU

            # W rows, i-partitioned (final matmul lhsT layout)
            Wg = pWgp.tile([128, ICH * SPC * T2], F32, tag="Wg")
            nc.vector.memset(Wg[:, :], 0.0)

            # ---------------- phase 4: backward W (row-staged) ----------------
            RSB = 4
            NGB = T1 // RSB
            with tc.tile_pool(name="ph4", bufs=1) as p4, \
                 tc.tile_pool(name="pms", bufs=2) as pms, \
                 tc.tile_pool(name="pws", bufs=2) as pws:

                def load_mask_stage(grid, g, pad_last):
                    i0 = g * RSB
                    c, p0 = i0 // 128, i0 % 128
                    ww = RSB * T2 + (1 if pad_last else 0)
                    t = pms.tile([SPC, ww], BF16, tag="ms", name="ms", bufs=6)
                    if pad_last:
                        nc.vector.memset(t[:, RSB * T2:RSB * T2 + 1], 0.0)
                    for s in range(SPC):
                        tv = t[s:s + 1, 0:RSB * T2].rearrange(
                            "a (r c2) -> a r c2", r=RSB)
                        nc.sync.dma_start(
                            tv.opt(),
                            grid[p0:p0 + RSB,
                                 blk(c, s):blk(c, s) + T2].opt())
                    return t

                wprev = None
                up_hi = dg_hi = None
                for g in range(NGB - 1, -1, -1):
                    i0 = g * RSB
                    lst = load_mask_stage(Mlg, g, pad_last=True)
                    ust = load_mask_stage(Mug, g, pad_last=False)
                    dst_ = load_mask_stage(Mdg, g, pad_last=False)
                    ws_ = pws.tile([SPC, RSB * (T2 + 1)], F32, tag="ws", name="ws")
                    for r in range(RSB - 1, -1, -1):
                        i = i0 + r
                        rb = r * (T2 + 1)
                        nc.vector.memset(ws_[:, rb + T2:rb + T2 + 1], 0.0)
                        wr = ws_[:, rb:rb + T2 + 1]
                        g_ = p4.tile([SPC, T2], BF16, tag="g")
                        if i == T1 - 1:
                            nc.vector.memset(g_[:, :], 0.0)
                            nc.vector.memset(g_[:, T2 - 1:T2], 1.0)
                        else:
                            if r == RSB - 1:
                                un = up_hi[:, 0:T2]
                                dn = dg_hi[:, 0:T2]
                            else:
                                un = ust[:, (r + 1) * T2:(r + 2) * T2]
                                dn = dst_[:, (r + 1) * T2:(r + 2) * T2]
                            nc.vector.tensor_tensor(
                                out=g_[:, :], in0=wprev[:, 0:T2], in1=un,
                                op=AOT.logical_and)
                            t2_ = p4.tile([SPC, T2 - 1], BF16, tag="t2")
                            nc.vector.tensor_tensor(
                                out=t2_[:, :], in0=wprev[:, 1:T2],
                                in1=dn[:, 1:T2], op=AOT.logical_and)
                            nc.vector.tensor_tensor(
                                out=g_[:, 0:T2 - 1], in0=g_[:, 0:T2 - 1],
                                in1=t2_[:, :], op=AOT.logical_or)
                        # W[i,j] = G[i,j] | (left[i,j+1] & W[i,j+1]), j desc.
                        nc.vector.tensor_tensor_scan(
                            out=wr[:, 0:T2][:, ::-1],
                            data0=lst[:, r * T2 + 1:(r + 1) * T2 + 1][:, ::-1],
                            data1=g_[:, ::-1], initial=0.0,
                            op0=AOT.logical_and, op1=AOT.logical_or)
                        wprev = wr
                    c, p0 = i0 // 128, i0 % 128
                    for s in range(SPC):
                        nc.sync.dma_start(
                            Wg[p0:p0 + RSB, (c * SPC + s) * T2:
                               (c * SPC + s + 1) * T2].opt(),
                            ws_[s:s + 1, :].rearrange("a (r c2) -> a r c2", r=RSB)
                            [:, :, 0:T2].opt())
                    up_hi = ust[:, 0:T2]
                    dg_hi = dst_[:, 0:T2]

            if dbg is not None:
                with tc.tile_pool(name="dbgp", bufs=1) as dp_:
                    for nm_, grid_ in (("Mdg", Mdg), ("Mug", Mug), ("Mlg", Mlg)):
                        tmp_ = dp_.tile([128, GW2], F32, tag="dtmp", name="dtmp")
                        nc.vector.tensor_copy(out=tmp_[:, :], in_=grid_[:, :])
                        nc.sync.dma_start(dbg[nm_][:, :], tmp_[:, :])
                nc.sync.dma_start(dbg["Wg"][:, :], Wg[:, :])
            esM.close()   # free mask grids

            # ---------------- phase 5: out = W^T @ teacher ----------------
            with tc.tile_pool(name="pp5", bufs=2, space="PSUM") as pp5, \
                 tc.tile_pool(name="p5s", bufs=3) as p5s:
                Wg2 = pWgp.tile([128, ICH * SPC * T2], F32, tag="Wg2")
                nc.vector.tensor_copy(out=Wg2[:, :], in_=Wg[:, :])
                tn = []
                for s in range(SPC):
                    tnr = p5s.tile([128, ICH * DM], F32, tag="tnr", name="tnr",
                                   bufs=2)
                    for c in range(ICH):
                        h = _ich(c)
                        nc.sync.dma_start(
                            tnr[0:h, c * DM:(c + 1) * DM],
                            tnat[s, 128 * c:128 * c + h, :])
                    tnc = p5s.tile([128, ICH * DM], F32, tag=f"tn{s}",
                                   name=f"tn{s}")
                    for c in range(ICH):
                        h = _ich(c)
                        nc.vector.tensor_copy(
                            out=tnc[0:h, c * DM:(c + 1) * DM],
                            in_=tnr[0:h, c * DM:(c + 1) * DM])
                    tn.append(tnc)
                for s in range(SPC):
                    for jm in range(T2 // MT):          # 5 M-tiles of 120
                        for n2 in range(DM // 512):     # 2 N-tiles of 512
                            ps = pp5.tile([MT, 512], F32, tag="ps5")
                            for c in range(ICH):
                                h = _ich(c)
                                wb = (c * SPC + s) * T2 + jm * MT
                                nc.tensor.matmul(
                                    ps[:, :],
                                    lhsT=Wg2[0:h, wb:wb + MT],
                                    rhs=tn[s][0:h, c * DM + 512 * n2:
                                              c * DM + 512 * (n2 + 1)],
                                    start=(c == 0), stop=(c == ICH - 1))
                            ob = p5s.tile([MT, 512], F32, tag="ob")
                            nc.vector.tensor_copy(out=ob[:, :], in_=ps[:, :])
                            nc.sync.dma_start(
                                out[s, jm * MT:(jm + 1) * MT,
                                    512 * n2:512 * (n2 + 1)], ob[:, :])
    return nc


_CACHE = {}


def _get_nc():
    if "nc" not in _CACHE:
        nc = bacc.Bacc("TRN2", target_bir_lowering=False, debug=False)
        build_kernel(nc)
        nc.finalize()
        _CACHE["nc"] = nc
    return _CACHE["nc"]


def kernel(teacher_features: np.ndarray, student_features: np.ndarray) -> np.ndarray:
    from concourse.bass_utils import run_bass_kernel_spmd

    t = np.ascontiguousarray(np.asarray(teacher_features, dtype=np.float32))
    s = np.ascontiguousarray(np.asarray(student_features, dtype=np.float32))
    nc = _get_nc()
    in_maps = []
    for c in range(NCORES):
        tc_ = t[SPC * c:SPC * (c + 1)]
        sc_ = s[SPC * c:SPC * (c + 1)]
        in_maps.append({
            "tT": np.ascontiguousarray(tc_.transpose(0, 2, 1)),
            "sT": np.ascontiguousarray(sc_.transpose(0, 2, 1)),
            "tnat": tc_,
        })
    res = run_bass_kernel_spmd(nc, in_maps, core_ids=list(range(NCORES)))
    return np.concatenate([res.results[c]["out"] for c in range(NCORES)], axis=0)

